# revision 33
# baseline (speedup 1.0000x reference)
"""3-layer GCN on a fixed 96x96 8-connected grid (quirky boundaries), Trainium2 Bass kernel.

Math: the reference's graph aggregation is D^-1/2 (A+I) D^-1/2 with A+I = R (x) C
(Kronecker-separable over grid rows/cols, including the reference's boundary
masking quirk), and the degree is separable too. So per layer:
    h' = relu(ds o (T (ds o h)) W + b),  ds = dsR (x) dsC,  T = Tr (x) Tc
where Tr/Tc are unweighted 3-tap sums with the quirky boundary:
    sources(t) = {t} + {t+1 if t<=94} + {t-1 if t>=2} + {95 if t==0}

Device plan (1 sample per core, 8 cores; layout [channels on partitions, 9216 nodes free]):
  - host pre-scales x by ds; host applies the final ds scale
  - DVE: row 3-sum Tr via 2 big shifted adds (+-96 elems, bf16 2x mode) + fixups
  - PE:  col taps Tc (3 accumulating matmuls with +-1 free offsets + strided
         corner tap) fused with the feature matmul W (bf16), fp32 PSUM
  - ACT: relu evacuation PSUM->SBUF (bf16)
  - DVE: multiply by ds^2 (replicated) to produce next layer's scaled input
"""

import numpy as np
import ml_dtypes

H = W = 96
N = H * W  # 9216
B, CIN, HID, COUT = 8, 64, 128, 64
BF16 = ml_dtypes.bfloat16


def _axis_quirky(n):
    # 0/1 matrix of the per-axis quirky 3-tap sum (see module docstring)
    M = np.zeros((n, n), np.float32)
    for t in range(n):
        M[t, t] = 1.0
        if t <= n - 2:
            M[t, t + 1] = 1.0
        if t >= 2:
            M[t, t - 1] = 1.0
        if t == 0:
            M[t, n - 1] += 1.0
    return M


def _norm_vectors():
    degR = _axis_quirky(H).sum(axis=1)
    degC = _axis_quirky(W).sum(axis=1)
    dsR = 1.0 / np.sqrt(degR)
    dsC = 1.0 / np.sqrt(degC)
    return np.outer(dsR, dsC).ravel().astype(np.float32)  # [N]


_NC_CACHE = {}


def _build_bass(has_bias):
    import concourse.mybir as mybir
    from concourse import bacc
    from concourse.tile import TileContext

    fp32 = mybir.dt.float32
    bf16 = mybir.dt.bfloat16
    RELU = mybir.ActivationFunctionType.Relu
    MULT = mybir.AluOpType.mult

    nc = bacc.Bacc("TRN2", target_bir_lowering=False)

    xh = nc.dram_tensor("xh", [CIN, N], bf16, kind="ExternalInput")
    w1 = nc.dram_tensor("w1", [CIN, HID], bf16, kind="ExternalInput")
    w2 = nc.dram_tensor("w2", [HID, HID], bf16, kind="ExternalInput")
    w3 = nc.dram_tensor("w3", [HID, COUT], bf16, kind="ExternalInput")
    ds2r = nc.dram_tensor("ds2r", [128, N], bf16, kind="ExternalInput")
    if has_bias:
        bcols = nc.dram_tensor("bcols", [1, HID * 3], bf16, kind="ExternalInput")
        invdsr = nc.dram_tensor("invdsr", [1, N], bf16, kind="ExternalInput")
    out = nc.dram_tensor("out", [COUT, N], bf16, kind="ExternalOutput")

    CHUNK_ROWS = 5  # 5 grid rows = 480 cols per matmul chunk (<=512 psum bank)
    CHUNK = CHUNK_ROWS * W
    GRP = 4  # psum banks per tile
    n_chunks_last = 19  # the single-row chunk (row 95)

    with TileContext(nc) as tc:
        with (
            tc.tile_pool(name="persist", bufs=1) as persist,
            tc.tile_pool(name="acts", bufs=2) as acts,
            tc.tile_pool(name="sbufs", bufs=2) as spool,
            tc.tile_pool(name="psum", bufs=2, space="PSUM") as pp,
        ):
            h0 = persist.tile([CIN, N], bf16, tag="h0")
            wt = [
                persist.tile([CIN, HID], bf16, tag="w1t", name="w1t"),
                persist.tile([HID, HID], bf16, tag="w2t", name="w2t"),
                persist.tile([HID, COUT], bf16, tag="w3t", name="w3t"),
            ]
            ds2 = persist.tile([128, N], bf16, tag="ds2")

            for q in range(4):
                sl = slice(q * (N // 4), (q + 1) * (N // 4))
                nc.sync.dma_start(h0[:, sl], xh[:, sl])
            nc.sync.dma_start(wt[0][:, :], w1[:, :])
            nc.sync.dma_start(wt[1][:, :], w2[:, :])
            nc.sync.dma_start(wt[2][:, :], w3[:, :])
            nc.sync.dma_start(ds2[:, :], ds2r[:, :])
            if has_bias:
                bc = persist.tile([1, HID * 3], bf16, tag="bc")
                ivd = persist.tile([1, N], bf16, tag="ivd")
                nc.sync.dma_start(bc[:, :], bcols[:, :])
                nc.sync.dma_start(ivd[:, :], invdsr[:, :])

            layer_dims = [(CIN, HID), (HID, HID), (HID, COUT)]
            # The wrap edges (row 0 <- row 95, col 0 <- col 95) make output
            # row 0 depend on input row 95. To pipeline layers, process a
            # tail region (low rows) at the END of each layer and everything
            # else ascending, so layer l+1 chases layer l's frontier. The
            # tail grows by one chunk per layer (the wrap cone expands).
            def plan(li):
                T = 6 + 5 * li  # first main Tr target row
                bands = [(r, min(r + 12, H)) for r in range(T, H, 12)] + [(0, T)]
                first_main = li + 2  # first chunk whose rows are all >= T
                main = list(range(first_main, 20))
                groups = [main[i : i + 4] for i in range(0, len(main), 4)]
                groups.append(list(range(first_main)))
                return bands, groups

            h_in = h0
            for li, (K, M) in enumerate(layer_dims):
                last = li == len(layer_dims) - 1
                v = nc.vector

                TR_BANDS, GROUPS = plan(li)

                # ---- Tr row-sum: s[r] = h[r] + h[r+1](r<=94) + h[r-1](r>=2) + h[95](r==0)
                s = spool.tile([K, N], bf16, tag="s")
                s3 = s.rearrange("p (r c) -> p r c", c=W)
                for r0, r1 in TR_BANDS:
                    a1 = min(r1, 95)  # self+down targets r<=94
                    if r0 < a1:
                        v.tensor_add(
                            s[:, r0 * W : a1 * W],
                            h_in[:, r0 * W : a1 * W],
                            h_in[:, (r0 + 1) * W : (a1 + 1) * W],
                        )
                    if r1 == 96:
                        v.tensor_copy(s[:, 95 * W : N], h_in[:, 95 * W : N])
                    b0 = max(r0, 2)  # +up targets r>=2
                    v.tensor_add(
                        s[:, b0 * W : r1 * W],
                        s[:, b0 * W : r1 * W],
                        h_in[:, (b0 - 1) * W : (r1 - 1) * W],
                    )
                    if r0 == 0:  # row-0 wrap: s[0] += h[95]
                        v.tensor_add(s[:, 0:W], s[:, 0:W], h_in[:, 95 * W : N])
                    # fold the Tc wrap (c'=0 <- c=95) into column 0 of s so the
                    # center tap matmul picks it up (no other tap reads col 0)
                    v.tensor_add(
                        s3[:, r0:r1, 0:1], s3[:, r0:r1, 0:1], s3[:, r0:r1, W - 1 : W]
                    )

                if last:
                    h_out = persist.tile([COUT, N], bf16, tag="hout")
                else:
                    h_out = acts.tile([M, N], bf16, tag="h")

                # ---- Tc taps folded into the feature matmul, chunked over nodes
                wT = wt[li][:, :]
                mm = nc.tensor.matmul
                for gi, chunks in enumerate(GROUPS):
                    ps = pp.tile([M, GRP * 512], fp32, tag="ps")
                    for b, ci in enumerate(chunks):
                        r0 = ci * CHUNK_ROWS
                        nr = min(CHUNK_ROWS, H - r0)
                        L = nr * W
                        n0 = r0 * W
                        pc = ps[:, b * 512 : b * 512 + L]
                        pc3 = pc.rearrange("p (r c) -> p r c", c=W)
                        mm(pc, wT, s[:, n0 : n0 + L], start=True, stop=False)
                        mm(
                            pc3[:, :, 0 : W - 1],
                            wT,
                            s3[:, r0 : r0 + nr, 1:W],
                            start=False,
                            stop=False,
                        )
                        mm(
                            pc3[:, :, 2:W],
                            wT,
                            s3[:, r0 : r0 + nr, 1 : W - 1],
                            start=False,
                            stop=not has_bias,
                        )
                        if has_bias:
                            mm(
                                pc,
                                bc[:, li * HID : li * HID + M],
                                ivd[:, n0 : n0 + L],
                                start=False,
                                stop=True,
                            )
                    # grouped relu evacuation (one ACT op per run of full chunks)
                    lo = chunks[0] * CHUNK
                    hi = min(N, (chunks[-1] + 1) * CHUNK)
                    psg = ps.rearrange("p (b k) -> p b k", k=512)
                    nfull = sum(1 for ci in chunks if ci != n_chunks_last)
                    if nfull:
                        nc.scalar.activation(
                            h_out[:, lo : lo + nfull * CHUNK],
                            psg[:, 0:nfull, 0:CHUNK],
                            RELU,
                        )
                    if nfull != len(chunks):  # group ends with 1-row chunk 19
                        nc.scalar.activation(
                            h_out[:, 95 * W : N],
                            ps[:, nfull * 512 : nfull * 512 + W],
                            RELU,
                        )
                    if last:
                        # store to DRAM; host applies the final ds scale
                        nc.sync.dma_start(out[:, lo:hi], h_out[:, lo:hi])
                    else:
                        # next-layer scaled input for this group's rows
                        # (alternate DVE / GPSIMD to keep DVE off the critical path)
                        eng = v if gi % 2 == 0 else nc.gpsimd
                        eng.tensor_tensor(
                            h_out[:, lo:hi], h_out[:, lo:hi], ds2[:M, lo:hi], MULT
                        )
                if not last:
                    h_in = h_out

    nc.finalize()
    return nc


def kernel(x, W1, b1, W2, b2, W3, b3, **_ignored):
    from concourse.bass_utils import run_bass_kernel_spmd

    ds = _norm_vectors()
    has_bias = bool(np.any(b1) or np.any(b2) or np.any(b3))

    key = has_bias
    if key not in _NC_CACHE:
        _NC_CACHE[key] = _build_bass(has_bias)
    nc = _NC_CACHE[key]

    xs = np.asarray(x, np.float32).reshape(B, CIN, N)
    xh = (xs * ds[None, None, :]).astype(BF16)
    ds2 = np.ascontiguousarray(np.broadcast_to((ds * ds).astype(BF16)[None, :], (128, N)))
    base = {
        "w1": np.asarray(W1, np.float32).astype(BF16),
        "w2": np.asarray(W2, np.float32).astype(BF16),
        "w3": np.asarray(W3, np.float32).astype(BF16),
        "ds2r": ds2,
    }
    if has_bias:
        bcols = np.concatenate(
            [
                np.pad(np.asarray(b, np.float32), (0, HID - len(b)))
                for b in (b1, b2, b3)
            ]
        ).astype(BF16)[None, :]
        base["bcols"] = bcols
        base["invdsr"] = (1.0 / ds).astype(BF16)[None, :]

    in_maps = [dict(base, xh=np.ascontiguousarray(xh[b])) for b in range(B)]
    res = run_bass_kernel_spmd(nc, in_maps, core_ids=list(range(B)))
    outs = np.stack([r["out"] for r in res.results])  # [B, COUT, N] bf16
    full = outs.astype(np.float32) * ds[None, None, :]
    return full.reshape(B, COUT, H, W)
